# revision 24
# baseline (speedup 1.0000x reference)
"""GQA kernel for Trainium2, 8 NeuronCores.

Problem: nn_GroupQueryAttention — B=4, S=2048, E=2048, 16 heads / 4 groups,
d_head=128.  out = softmax((x@Wq) (x@Wk)^T / sqrt(d)) (x@Wv) @ Wo + biases.

Sharding: core c -> (batch b = c//2, half = c%2).  Each core handles one
batch and 2 of the 4 KV groups (= 8 of the 16 heads): Wq columns / Wo rows
split by head, Wk/Wv columns split by group.  Each core produces a partial
output projection for its batch; the host sums the two halves.

v2 design (all matmuls bf16, 1024-col moving operands so LDWEIGHTS hides
under streaming; phase order B,A,C,D with DMA prefetch):
  - inputs fed pre-transposed x^T [E,S] in bf16; weights bf16.
  - bk is DROPPED: a per-query-column constant in scores cancels in softmax
    (exact).  bv+bo are folded host-side into bo_eff = bo + bv_exp @ Wo_half
    (exact, since ctx = ctxu/rsum + bv after normalization).  bq is folded
    into qh during the PSUM->SBUF copy (per-partition scalar add).
  - B: qh^T[d,s] = Wq_h^T x^T  (stationary Wq chunks, moving x^T 1024-col)
  - A: kT[d,t] = Wk_g^T x^T; vhT[d,t] = Wv_g^T x^T then PE-transposed to
    vh[t,d] via identity matmuls.
  - C: per (head, q-half): s^T[t,q] (kT chunk stationary), ex = exp(s/sqrt d)
    on scalar engine (bf16), row-sums via DVE accumulation of ex chunks +
    gpsimd partition_all_reduce (no PE ones-matmuls), 1/rsum via
    reciprocal_approx_fast, ctx^T = (vh^T ex) * rr.
  - D: out[s,e] = sum_h cx_h^T.T @ Wo_h + bo_eff; out DMA per 128-row tile.
Softmax skips max-subtraction: scores ~N(0,1), far from fp32 exp overflow.
"""

import sys

sys.path.insert(0, "/opt/trn_rl_repo")

import numpy as np
import ml_dtypes

BF16 = ml_dtypes.bfloat16

B, S, E = 4, 2048, 2048
D = 128            # head dim
HPC = 8            # heads per core
GPC = 2            # groups per core
QC = HPC * D       # 1024 Wq cols per core
KV = GPC * D       # 256 Wk/Wv cols per core
NE = E // D        # 16 contraction chunks
NT = S // D        # 16 t-chunks of 128
QH = 2             # q halves of 1024
QW = S // QH       # 1024
N_CORES = 8

_PROGRAM = None


def _build():
    from contextlib import ExitStack

    import concourse.bass as bass
    import concourse.mybir as mybir
    import concourse.tile as tile
    from concourse import bacc, bass_isa

    F32 = mybir.dt.float32
    BF = mybir.dt.bfloat16
    Exp = mybir.ActivationFunctionType.Exp
    SCALE = 1.0 / float(np.sqrt(D))

    nc = bacc.Bacc("TRN2", target_bir_lowering=False, debug=False)
    xq = nc.dram_tensor("xq", [E, S], BF, kind="ExternalInput")
    xk = nc.dram_tensor("xk", [E, S], BF, kind="ExternalInput")
    xv = nc.dram_tensor("xv", [E, S], BF, kind="ExternalInput")
    wq = nc.dram_tensor("wq", [E, QC], BF, kind="ExternalInput")
    wk = nc.dram_tensor("wk", [E, KV], BF, kind="ExternalInput")
    wv = nc.dram_tensor("wv", [E, KV], BF, kind="ExternalInput")
    wo = nc.dram_tensor("wo", [QC, E], BF, kind="ExternalInput")
    bq = nc.dram_tensor("bq", [QC], F32, kind="ExternalInput")
    bo = nc.dram_tensor("bo", [E], F32, kind="ExternalInput")
    ident = nc.dram_tensor("ident", [D, D], BF, kind="ExternalInput")
    out = nc.dram_tensor("out_p", [S, E], F32, kind="ExternalOutput")

    wq_r = wq.ap().rearrange("(n p) c -> p n c", p=D)   # [128,16,1024]
    wo_r = wo.ap().rearrange("(h p) e -> p h e", p=D)   # [128,8,2048]

    def bcast(dram, n):
        return bass.AP(tensor=dram.ap().tensor, offset=0, ap=[[0, D], [1, n]])

    with tile.TileContext(nc) as tc:
        with ExitStack() as top:
            const = top.enter_context(tc.tile_pool(name="const", bufs=1))
            acts = top.enter_context(tc.tile_pool(name="acts", bufs=1))

            bq_sb = const.tile([D, HPC], F32)
            nc.sync.dma_start(out=bq_sb, in_=bq.ap().rearrange("(h d) -> d h", d=D))
            ident_sb = const.tile([D, D], BF)
            nc.sync.dma_start(out=ident_sb, in_=ident.ap())
            ones_sb = const.tile([D, D], BF)
            nc.vector.memset(ones_sb, 1.0)

            # persistent activations
            kT = [acts.tile([D, S], BF, name=f"kT{g}") for g in range(GPC)]
            vh = [acts.tile([D, KV], BF, name=f"vh{t}") for t in range(NT)]

            def qcx_tile(name):
                return acts.tile([D, S], BF, name=name, tag="qcx", bufs=9)

            qh = []
            cx = []

            # ---- Phases B (Q proj) and A (K/V proj + transpose) ----
            with tc.tile_pool(name="pba", bufs=1) as pba:
                def xch(name):
                    return pba.tile([D, QW], BF, name=name, tag="xh", bufs=48)

                # xq split into s-halves: half0 on sync (feeds B sh=0
                # immediately), half1 on gpsimd queue; wq + K/V weights + xv
                # on the scalar queue; xk behind xq-half0 on sync.
                wq_sb = [pba.tile([D, QC], BF, name=f"wq{nn}") for nn in range(NE)]
                xq_ch = [
                    [xch(f"xq{e}_{hf}") for hf in range(2)] for e in range(NE)
                ]
                for e in range(NE):
                    nc.scalar.dma_start(out=wq_sb[e], in_=wq_r[:, e, :])
                    nc.sync.dma_start(
                        out=xq_ch[e][0],
                        in_=xq.ap()[e * D : (e + 1) * D, 0:QW],
                    )
                    nc.gpsimd.dma_start(
                        out=xq_ch[e][1],
                        in_=xq.ap()[e * D : (e + 1) * D, QW:S],
                    )
                wk_sb = pba.tile([D, NE, KV], BF)
                nc.scalar.dma_start(
                    out=wk_sb, in_=wk.ap().rearrange("(n p) c -> p n c", p=D)
                )
                wv_sb = pba.tile([D, NE, KV], BF)
                nc.scalar.dma_start(
                    out=wv_sb, in_=wv.ap().rearrange("(n p) c -> p n c", p=D)
                )
                xk_ch = []
                for e in range(NE):
                    hfs = []
                    for hf in range(2):
                        t_ = xch(f"xk{e}_{hf}")
                        nc.sync.dma_start(
                            out=t_,
                            in_=xk.ap()[e * D : (e + 1) * D, hf * QW : (hf + 1) * QW],
                        )
                        hfs.append(t_)
                    xk_ch.append(hfs)

                # ---- Phase B compute ----
                # 8 single-bank accumulators (one per head) per s-quarter:
                # 4 boundaries instead of 12, finer DMA gating; FWL-fast
                # LDWEIGHTS (~97ns) hides under 213ns matmuls even with a
                # fresh stationary per matmul.
                with tc.tile_pool(name="psb", bufs=1, space="PSUM") as psb:
                    for h in range(HPC):
                        qh.append(qcx_tile(f"qh{h}"))
                    for sq in range(4):
                        sh, qq = sq // 2, sq % 2
                        ps_q = [
                            psb.tile(
                                [D, 512], F32, name=f"psq{sq}_{i}",
                                tag="psq", bufs=8,
                            )
                            for i in range(HPC)
                        ]
                        for e in range(NE):
                            for h in range(HPC):
                                nc.tensor.matmul(
                                    ps_q[h],
                                    wq_sb[e][:, h * D : (h + 1) * D],
                                    xq_ch[e][sh][:, qq * 512 : (qq + 1) * 512],
                                    start=(e == 0),
                                    stop=(e == NE - 1),
                                )
                        for h in range(HPC):
                            nc.vector.tensor_scalar_add(
                                out=qh[h][:, sq * 512 : (sq + 1) * 512],
                                in0=ps_q[h],
                                scalar1=bq_sb[:, h : h + 1],
                            )


                # ---- Phase A ----
                # xv chunks queue behind (land as xq slots free at B end)
                xv_ch = []
                for e in range(NE):
                    hfs = []
                    for hf in range(2):
                        t_ = xch(f"xv{e}_{hf}")
                        nc.scalar.dma_start(
                            out=t_,
                            in_=xv.ap()[e * D : (e + 1) * D, hf * QW : (hf + 1) * QW],
                        )
                        hfs.append(t_)
                    xv_ch.append(hfs)

                with tc.tile_pool(name="psA", bufs=1, space="PSUM") as psA:
                    # K projection: per (group, s-half), stationary reused x2
                    for g in range(GPC):
                        for jj in range(2):
                            ps_k = psA.tile(
                                [D, QW], F32, name=f"psk{g}_{jj}",
                                tag="psa", bufs=3,
                            )
                            for e in range(NE):
                                for qq in range(2):
                                    nc.tensor.matmul(
                                        ps_k[:, qq * 512 : (qq + 1) * 512],
                                        wk_sb[:, e, g * D : (g + 1) * D],
                                        xk_ch[e][jj][:, qq * 512 : (qq + 1) * 512],
                                        start=(e == 0),
                                        stop=(e == NE - 1),
                                    )
                            nc.vector.tensor_copy(
                                out=kT[g][:, jj * QW : (jj + 1) * QW], in_=ps_k
                            )
                    # V projection (as vhT) then PE-transpose to vh
                    vhT = []
                    for g in range(GPC):
                        vt = pba.tile([D, S], BF, name=f"vhT{g}", tag="vhT", bufs=2)
                        for jj in range(2):
                            ps_v = psA.tile(
                                [D, QW], F32, name=f"psv{g}_{jj}",
                                tag="psa", bufs=3,
                            )
                            for e in range(NE):
                                for qq in range(2):
                                    nc.tensor.matmul(
                                        ps_v[:, qq * 512 : (qq + 1) * 512],
                                        wv_sb[:, e, g * D : (g + 1) * D],
                                        xv_ch[e][jj][:, qq * 512 : (qq + 1) * 512],
                                        start=(e == 0),
                                        stop=(e == NE - 1),
                                    )
                            nc.vector.tensor_copy(
                                out=vt[:, jj * QW : (jj + 1) * QW], in_=ps_v
                            )
                        vhT.append(vt)
                    for g in range(GPC):
                        for t in range(NT):
                            ps_t = psA.tile(
                                [D, D], BF, name=f"pst{g}_{t}", tag="pst", bufs=2
                            )
                            nc.tensor.transpose(
                                ps_t, vhT[g][:, t * D : (t + 1) * D], ident_sb
                            )
                            nc.vector.tensor_copy(
                                out=vh[t][:, g * D : (g + 1) * D], in_=ps_t
                            )

            # ---- Phases C (attention) and D (output projection) ----
            with (
                tc.tile_pool(name="pc", bufs=1) as pc,
                tc.tile_pool(name="pd", bufs=1) as pd,
            ):
                # prefetch D-phase weights during C
                wo_sb = pd.tile([D, HPC, E], BF)
                nc.sync.dma_start(out=wo_sb, in_=wo_r)
                bo_rep = pd.tile([D, E], F32)
                nc.sync.dma_start(out=bo_rep, in_=bcast(bo, E))

                with tc.tile_pool(name="psc", bufs=1, space="PSUM") as psc:
                    for h in range(HPC):
                        g = h // (HPC // GPC)
                        cxt = qcx_tile(f"cx{h}")
                        cx.append(cxt)
                        for q2 in range(QH):
                            qsl = qh[h][:, q2 * QW : (q2 + 1) * QW]
                            ps_ctx = psc.tile(
                                [D, QW], F32, name=f"psctx{h}_{q2}",
                                tag="psctx", bufs=2,
                            )
                            acc = pc.tile(
                                [D, QW], BF, name=f"acc{h}_{q2}", tag="acc", bufs=2
                            )
                            # software-pipelined by one t-step: scores(t+1)
                            # is emitted before ctx(t) so the PE does not
                            # wait on the scalar engine's exp
                            ps_s = []
                            ex = []

                            def emit_scores(t):
                                p = psc.tile(
                                    [D, QW], F32, name=f"pss{h}_{q2}_{t}",
                                    tag="pss", bufs=2,
                                )
                                for j in range(2):
                                    nc.tensor.matmul(
                                        p[:, j * 512 : (j + 1) * 512],
                                        kT[g][:, t * D : (t + 1) * D],
                                        qsl[:, j * 512 : (j + 1) * 512],
                                        start=True, stop=True,
                                    )
                                ps_s.append(p)

                            def emit_exp_acc(t):
                                x_ = pc.tile(
                                    [D, QW], BF, name=f"ex{h}_{q2}_{t}",
                                    tag="ex", bufs=6,
                                )
                                nc.scalar.activation(
                                    out=x_, in_=ps_s[t], func=Exp, scale=SCALE
                                )
                                ex.append(x_)
                                if t == 0:
                                    pass
                                elif t == 1:
                                    nc.vector.tensor_add(out=acc, in0=ex[0], in1=x_)
                                else:
                                    nc.vector.tensor_add(out=acc, in0=acc, in1=x_)

                            emit_scores(0)
                            for t in range(NT):
                                if t + 1 < NT:
                                    emit_scores(t + 1)
                                emit_exp_acc(t)
                                for j in range(2):
                                    nc.tensor.matmul(
                                        ps_ctx[:, j * 512 : (j + 1) * 512],
                                        vh[t][:, g * D : (g + 1) * D],
                                        ex[t][:, j * 512 : (j + 1) * 512],
                                        start=(t == 0),
                                        stop=(t == NT - 1),
                                    )
                            ps_rs = psc.tile(
                                [D, QW], F32, name=f"psrs{h}_{q2}",
                                tag="psctx", bufs=2,
                            )
                            for j in range(2):
                                nc.tensor.matmul(
                                    ps_rs[:, j * 512 : (j + 1) * 512],
                                    ones_sb,
                                    acc[:, j * 512 : (j + 1) * 512],
                                    start=True, stop=True,
                                )
                            rr = pc.tile(
                                [D, QW], F32, name=f"rr{h}_{q2}", tag="rr", bufs=2
                            )
                            nc.vector.reciprocal_approx_fast(out=rr, in_=ps_rs)
                            nc.vector.tensor_mul(
                                out=cxt[:, q2 * QW : (q2 + 1) * QW],
                                in0=ps_ctx,
                                in1=rr,
                            )

                # ---- Phase D ----
                with tc.tile_pool(name="psd", bufs=1, space="PSUM") as psd:
                    for ss in range(NT):
                        ps_o = psd.tile(
                            [D, E], F32, name=f"pso{ss}", tag="pso", bufs=2
                        )
                        for hh in range(HPC):
                            for eh in range(4):
                                nc.tensor.matmul(
                                    ps_o[:, eh * 512 : (eh + 1) * 512],
                                    cx[hh][:, ss * D : (ss + 1) * D],
                                    wo_sb[:, hh, eh * 512 : (eh + 1) * 512],
                                    start=(hh == 0),
                                    stop=(hh == HPC - 1),
                                )
                        # last two row-tiles drain in 512-col quarters on
                        # alternating queues to shorten the serial tail
                        nq = 4 if ss >= NT - 2 else 2
                        w_ = E // nq
                        for oh in range(nq):
                            ot = pd.tile(
                                [D, QW], F32, name=f"ot{ss}_{oh}", tag="ot", bufs=4
                            )
                            nc.vector.tensor_add(
                                out=ot[:, 0:w_],
                                in0=ps_o[:, oh * w_ : (oh + 1) * w_],
                                in1=bo_rep[:, oh * w_ : (oh + 1) * w_],
                            )
                            eng = nc.sync if oh % 2 == 0 else nc.scalar
                            eng.dma_start(
                                out=out.ap()[
                                    ss * D : (ss + 1) * D, oh * w_ : (oh + 1) * w_
                                ],
                                in_=ot[:, 0:w_],
                            )

    nc.compile()
    return nc


def _get_program():
    global _PROGRAM
    if _PROGRAM is None:
        _PROGRAM = _build()
    return _PROGRAM


def make_in_maps(q, k, v, Wq, bq, Wk, bk, Wv, bv, Wo, bo):
    f32 = lambda a: np.asarray(a, dtype=np.float32)
    q, k, v = f32(q), f32(k), f32(v)
    Wq, bq, Wk, bk, Wv, bv, Wo, bo = (
        f32(Wq), f32(bq), f32(Wk), f32(bk), f32(Wv), f32(bv), f32(Wo), f32(bo)
    )
    ident = np.eye(D, dtype=BF16)
    in_maps = []
    xT = {}
    for b in range(B):
        xT[b] = (
            np.ascontiguousarray(q[b].T).astype(BF16),
            np.ascontiguousarray(k[b].T).astype(BF16),
            np.ascontiguousarray(v[b].T).astype(BF16),
        )
    halves = []
    for half in range(2):
        Wo_half = Wo[half * QC : (half + 1) * QC, :]
        bv_half = bv[half * KV : (half + 1) * KV]
        bv_exp = np.concatenate(
            [bv_half[(j // 4) * D : (j // 4 + 1) * D] for j in range(HPC)]
        )
        bo_eff = (bo if half == 0 else np.zeros_like(bo)).astype(
            np.float64
        ) + bv_exp.astype(np.float64) @ Wo_half.astype(np.float64)
        halves.append(
            {
                "wq": np.ascontiguousarray(
                    Wq[:, half * QC : (half + 1) * QC]
                ).astype(BF16),
                "wk": np.ascontiguousarray(
                    Wk[:, half * KV : (half + 1) * KV]
                ).astype(BF16),
                "wv": np.ascontiguousarray(
                    Wv[:, half * KV : (half + 1) * KV]
                ).astype(BF16),
                "wo": np.ascontiguousarray(Wo_half).astype(BF16),
                "bq": np.ascontiguousarray(bq[half * QC : (half + 1) * QC]),
                "bo": bo_eff.astype(np.float32),
                "ident": ident,
            }
        )
    for c in range(N_CORES):
        b, half = c // 2, c % 2
        xqT, xkT, xvT = xT[b]
        in_maps.append({"xq": xqT, "xk": xkT, "xv": xvT, **halves[half]})
    return in_maps


def combine_results(results):
    out = np.empty((B, S, E), np.float32)
    for b in range(B):
        out[b] = np.asarray(results[2 * b]["out_p"]) + np.asarray(
            results[2 * b + 1]["out_p"]
        )
    return out


def kernel(q, k, v, Wq, bq, Wk, bk, Wv, bv, Wo, bo):
    from concourse.bass_utils import run_bass_kernel_spmd

    nc = _get_program()
    in_maps = make_in_maps(q, k, v, Wq, bq, Wk, bk, Wv, bv, Wo, bo)
    res = run_bass_kernel_spmd(nc, in_maps, core_ids=list(range(N_CORES)))
    return combine_results(res.results)


# revision 25
# speedup vs baseline: 1.0087x; 1.0087x over previous
"""GQA kernel for Trainium2, 8 NeuronCores.

Problem: nn_GroupQueryAttention — B=4, S=2048, E=2048, 16 heads / 4 groups,
d_head=128.  out = softmax((x@Wq) (x@Wk)^T / sqrt(d)) (x@Wv) @ Wo + biases.

Sharding: core c -> (batch b = c//2, half = c%2).  Each core handles one
batch and 2 of the 4 KV groups (= 8 of the 16 heads): Wq columns / Wo rows
split by head, Wk/Wv columns split by group.  Each core produces a partial
output projection for its batch; the host sums the two halves.

v2 design (all matmuls bf16, 1024-col moving operands so LDWEIGHTS hides
under streaming; phase order B,A,C,D with DMA prefetch):
  - inputs fed pre-transposed x^T [E,S] in bf16; weights bf16.
  - bk is DROPPED: a per-query-column constant in scores cancels in softmax
    (exact).  bv+bo are folded host-side into bo_eff = bo + bv_exp @ Wo_half
    (exact, since ctx = ctxu/rsum + bv after normalization).  bq is folded
    into qh during the PSUM->SBUF copy (per-partition scalar add).
  - B: qh^T[d,s] = Wq_h^T x^T  (stationary Wq chunks, moving x^T 1024-col)
  - A: kT[d,t] = Wk_g^T x^T; vhT[d,t] = Wv_g^T x^T then PE-transposed to
    vh[t,d] via identity matmuls.
  - C: per (head, q-half): s^T[t,q] (kT chunk stationary), ex = exp(s/sqrt d)
    on scalar engine (bf16), row-sums via DVE accumulation of ex chunks +
    gpsimd partition_all_reduce (no PE ones-matmuls), 1/rsum via
    reciprocal_approx_fast, ctx^T = (vh^T ex) * rr.
  - D: out[s,e] = sum_h cx_h^T.T @ Wo_h + bo_eff; out DMA per 128-row tile.
Softmax skips max-subtraction: scores ~N(0,1), far from fp32 exp overflow.
"""

import sys

sys.path.insert(0, "/opt/trn_rl_repo")

import numpy as np
import ml_dtypes

BF16 = ml_dtypes.bfloat16

B, S, E = 4, 2048, 2048
D = 128            # head dim
HPC = 8            # heads per core
GPC = 2            # groups per core
QC = HPC * D       # 1024 Wq cols per core
KV = GPC * D       # 256 Wk/Wv cols per core
NE = E // D        # 16 contraction chunks
NT = S // D        # 16 t-chunks of 128
QH = 2             # q halves of 1024
QW = S // QH       # 1024
N_CORES = 8

_PROGRAM = None


def _build():
    from contextlib import ExitStack

    import concourse.bass as bass
    import concourse.mybir as mybir
    import concourse.tile as tile
    from concourse import bacc, bass_isa

    F32 = mybir.dt.float32
    BF = mybir.dt.bfloat16
    Exp = mybir.ActivationFunctionType.Exp
    SCALE = 1.0 / float(np.sqrt(D))

    nc = bacc.Bacc("TRN2", target_bir_lowering=False, debug=False)
    xq = nc.dram_tensor("xq", [E, S], BF, kind="ExternalInput")
    xk = nc.dram_tensor("xk", [E, S], BF, kind="ExternalInput")
    xv = nc.dram_tensor("xv", [E, S], BF, kind="ExternalInput")
    wq = nc.dram_tensor("wq", [E, QC], BF, kind="ExternalInput")
    wk = nc.dram_tensor("wk", [E, KV], BF, kind="ExternalInput")
    wv = nc.dram_tensor("wv", [E, KV], BF, kind="ExternalInput")
    wo = nc.dram_tensor("wo", [QC, E], BF, kind="ExternalInput")
    bq = nc.dram_tensor("bq", [QC], F32, kind="ExternalInput")
    bo = nc.dram_tensor("bo", [E], F32, kind="ExternalInput")
    ident = nc.dram_tensor("ident", [D, D], BF, kind="ExternalInput")
    out = nc.dram_tensor("out_p", [S, E], F32, kind="ExternalOutput")

    wq_r = wq.ap().rearrange("(n p) c -> p n c", p=D)   # [128,16,1024]
    wo_r = wo.ap().rearrange("(h p) e -> p h e", p=D)   # [128,8,2048]

    def bcast(dram, n):
        return bass.AP(tensor=dram.ap().tensor, offset=0, ap=[[0, D], [1, n]])

    with tile.TileContext(nc) as tc:
        with ExitStack() as top:
            const = top.enter_context(tc.tile_pool(name="const", bufs=1))
            acts = top.enter_context(tc.tile_pool(name="acts", bufs=1))

            bq_sb = const.tile([D, HPC], F32)
            nc.sync.dma_start(out=bq_sb, in_=bq.ap().rearrange("(h d) -> d h", d=D))
            ident_sb = const.tile([D, D], BF)
            nc.sync.dma_start(out=ident_sb, in_=ident.ap())
            ones_sb = const.tile([D, D], BF)
            nc.vector.memset(ones_sb, 1.0)

            # persistent activations
            kT = [acts.tile([D, S], BF, name=f"kT{g}") for g in range(GPC)]
            vh = [acts.tile([D, KV], BF, name=f"vh{t}") for t in range(NT)]

            def qcx_tile(name):
                return acts.tile([D, S], BF, name=name, tag="qcx", bufs=9)

            qh = []
            cx = []

            # ---- Phases B (Q proj) and A (K/V proj + transpose) ----
            with tc.tile_pool(name="pba", bufs=1) as pba:
                def xch(name):
                    return pba.tile([D, QW], BF, name=name, tag="xh", bufs=48)

                # xq split into s-halves: half0 on sync (feeds B sh=0
                # immediately), half1 on gpsimd queue; wq + K/V weights + xv
                # on the scalar queue; xk behind xq-half0 on sync.
                wq_sb = [pba.tile([D, QC], BF, name=f"wq{nn}") for nn in range(NE)]
                xq_ch = [
                    [xch(f"xq{e}_{hf}") for hf in range(2)] for e in range(NE)
                ]
                for e in range(NE):
                    nc.scalar.dma_start(out=wq_sb[e], in_=wq_r[:, e, :])
                    nc.sync.dma_start(
                        out=xq_ch[e][0],
                        in_=xq.ap()[e * D : (e + 1) * D, 0:QW],
                    )
                    nc.gpsimd.dma_start(
                        out=xq_ch[e][1],
                        in_=xq.ap()[e * D : (e + 1) * D, QW:S],
                    )
                wk_sb = pba.tile([D, NE, KV], BF)
                nc.scalar.dma_start(
                    out=wk_sb, in_=wk.ap().rearrange("(n p) c -> p n c", p=D)
                )
                wv_sb = pba.tile([D, NE, KV], BF)
                nc.scalar.dma_start(
                    out=wv_sb, in_=wv.ap().rearrange("(n p) c -> p n c", p=D)
                )
                xk_ch = []
                for e in range(NE):
                    hfs = []
                    for hf in range(2):
                        t_ = xch(f"xk{e}_{hf}")
                        nc.sync.dma_start(
                            out=t_,
                            in_=xk.ap()[e * D : (e + 1) * D, hf * QW : (hf + 1) * QW],
                        )
                        hfs.append(t_)
                    xk_ch.append(hfs)

                # ---- Phase B compute ----
                # 8 single-bank accumulators (one per head) per s-quarter:
                # 4 boundaries instead of 12, finer DMA gating; FWL-fast
                # LDWEIGHTS (~97ns) hides under 213ns matmuls even with a
                # fresh stationary per matmul.
                with tc.tile_pool(name="psb", bufs=1, space="PSUM") as psb:
                    for h in range(HPC):
                        qh.append(qcx_tile(f"qh{h}"))
                    for sq in range(4):
                        sh, qq = sq // 2, sq % 2
                        ps_q = [
                            psb.tile(
                                [D, 512], F32, name=f"psq{sq}_{i}",
                                tag="psq", bufs=8,
                            )
                            for i in range(HPC)
                        ]
                        for e in range(NE):
                            for h in range(HPC):
                                nc.tensor.matmul(
                                    ps_q[h],
                                    wq_sb[e][:, h * D : (h + 1) * D],
                                    xq_ch[e][sh][:, qq * 512 : (qq + 1) * 512],
                                    start=(e == 0),
                                    stop=(e == NE - 1),
                                )
                        for h in range(HPC):
                            nc.vector.tensor_scalar_add(
                                out=qh[h][:, sq * 512 : (sq + 1) * 512],
                                in0=ps_q[h],
                                scalar1=bq_sb[:, h : h + 1],
                            )


                # ---- Phase A ----
                # xv chunks queue behind (land as xq slots free at B end)
                xv_ch = []
                for e in range(NE):
                    hfs = []
                    for hf in range(2):
                        t_ = xch(f"xv{e}_{hf}")
                        nc.scalar.dma_start(
                            out=t_,
                            in_=xv.ap()[e * D : (e + 1) * D, hf * QW : (hf + 1) * QW],
                        )
                        hfs.append(t_)
                    xv_ch.append(hfs)

                with tc.tile_pool(name="psA", bufs=1, space="PSUM") as psA:
                    # K projection: per (group, s-half), stationary reused x2
                    for g in range(GPC):
                        for jj in range(2):
                            ps_k = psA.tile(
                                [D, QW], F32, name=f"psk{g}_{jj}",
                                tag="psa", bufs=3,
                            )
                            for e in range(NE):
                                for qq in range(2):
                                    nc.tensor.matmul(
                                        ps_k[:, qq * 512 : (qq + 1) * 512],
                                        wk_sb[:, e, g * D : (g + 1) * D],
                                        xk_ch[e][jj][:, qq * 512 : (qq + 1) * 512],
                                        start=(e == 0),
                                        stop=(e == NE - 1),
                                    )
                            nc.vector.tensor_copy(
                                out=kT[g][:, jj * QW : (jj + 1) * QW], in_=ps_k
                            )
                    # V projection (as vhT) then PE-transpose to vh
                    vhT = []
                    for g in range(GPC):
                        vt = pba.tile([D, S], BF, name=f"vhT{g}", tag="vhT", bufs=2)
                        for jj in range(2):
                            ps_v = psA.tile(
                                [D, QW], F32, name=f"psv{g}_{jj}",
                                tag="psa", bufs=3,
                            )
                            for e in range(NE):
                                for qq in range(2):
                                    nc.tensor.matmul(
                                        ps_v[:, qq * 512 : (qq + 1) * 512],
                                        wv_sb[:, e, g * D : (g + 1) * D],
                                        xv_ch[e][jj][:, qq * 512 : (qq + 1) * 512],
                                        start=(e == 0),
                                        stop=(e == NE - 1),
                                    )
                            nc.vector.tensor_copy(
                                out=vt[:, jj * QW : (jj + 1) * QW], in_=ps_v
                            )
                        vhT.append(vt)
                    for g in range(GPC):
                        for t in range(NT):
                            ps_t = psA.tile(
                                [D, D], BF, name=f"pst{g}_{t}", tag="pst", bufs=2
                            )
                            nc.tensor.transpose(
                                ps_t, vhT[g][:, t * D : (t + 1) * D], ident_sb
                            )
                            nc.vector.tensor_copy(
                                out=vh[t][:, g * D : (g + 1) * D], in_=ps_t
                            )

            # ---- Phases C (attention) and D (output projection) ----
            with (
                tc.tile_pool(name="pc", bufs=1) as pc,
                tc.tile_pool(name="pd", bufs=1) as pd,
            ):
                # prefetch D-phase weights during C
                wo_sb = pd.tile([D, HPC, E], BF)
                nc.sync.dma_start(out=wo_sb, in_=wo_r)
                bo_rep = pd.tile([D, E], F32)
                nc.sync.dma_start(out=bo_rep, in_=bcast(bo, E))

                with tc.tile_pool(name="psc", bufs=1, space="PSUM") as psc:
                    for h in range(HPC):
                        g = h // (HPC // GPC)
                        cxt = qcx_tile(f"cx{h}")
                        cx.append(cxt)
                        for q2 in range(QH):
                            qsl = qh[h][:, q2 * QW : (q2 + 1) * QW]
                            ps_ctx = psc.tile(
                                [D, QW], F32, name=f"psctx{h}_{q2}",
                                tag="psctx", bufs=2,
                            )
                            acc = pc.tile(
                                [D, QW], BF, name=f"acc{h}_{q2}", tag="acc", bufs=2
                            )
                            # software-pipelined by one t-step: scores(t+1)
                            # is emitted before ctx(t) so the PE does not
                            # wait on the scalar engine's exp
                            ps_s = []
                            ex = []

                            def emit_scores(t):
                                p = psc.tile(
                                    [D, QW], F32, name=f"pss{h}_{q2}_{t}",
                                    tag="pss", bufs=2,
                                )
                                for j in range(2):
                                    nc.tensor.matmul(
                                        p[:, j * 512 : (j + 1) * 512],
                                        kT[g][:, t * D : (t + 1) * D],
                                        qsl[:, j * 512 : (j + 1) * 512],
                                        start=True, stop=True,
                                    )
                                ps_s.append(p)

                            def emit_exp_acc(t):
                                x_ = pc.tile(
                                    [D, QW], BF, name=f"ex{h}_{q2}_{t}",
                                    tag="ex", bufs=6,
                                )
                                nc.scalar.activation(
                                    out=x_, in_=ps_s[t], func=Exp, scale=SCALE
                                )
                                ex.append(x_)
                                if t == 0:
                                    pass
                                elif t == 1:
                                    nc.vector.tensor_add(out=acc, in0=ex[0], in1=x_)
                                else:
                                    nc.vector.tensor_add(out=acc, in0=acc, in1=x_)

                            emit_scores(0)
                            for t in range(NT):
                                if t + 1 < NT:
                                    emit_scores(t + 1)
                                emit_exp_acc(t)
                                for j in range(2):
                                    nc.tensor.matmul(
                                        ps_ctx[:, j * 512 : (j + 1) * 512],
                                        vh[t][:, g * D : (g + 1) * D],
                                        ex[t][:, j * 512 : (j + 1) * 512],
                                        start=(t == 0),
                                        stop=(t == NT - 1),
                                    )
                            ps_rs = psc.tile(
                                [D, QW], F32, name=f"psrs{h}_{q2}",
                                tag="psctx", bufs=2,
                            )
                            for j in range(2):
                                nc.tensor.matmul(
                                    ps_rs[:, j * 512 : (j + 1) * 512],
                                    ones_sb,
                                    acc[:, j * 512 : (j + 1) * 512],
                                    start=True, stop=True,
                                )
                            rr = pc.tile(
                                [D, QW], F32, name=f"rr{h}_{q2}", tag="rr", bufs=2
                            )
                            nc.vector.reciprocal_approx_fast(out=rr, in_=ps_rs)
                            nc.vector.tensor_mul(
                                out=cxt[:, q2 * QW : (q2 + 1) * QW],
                                in0=ps_ctx,
                                in1=rr,
                            )

                # ---- Phase D ----
                with tc.tile_pool(name="psd", bufs=1, space="PSUM") as psd:
                    for ss in range(NT):
                        ps_o = psd.tile(
                            [D, E], F32, name=f"pso{ss}", tag="pso", bufs=2
                        )
                        for hh in range(HPC):
                            for eh in range(4):
                                nc.tensor.matmul(
                                    ps_o[:, eh * 512 : (eh + 1) * 512],
                                    cx[hh][:, ss * D : (ss + 1) * D],
                                    wo_sb[:, hh, eh * 512 : (eh + 1) * 512],
                                    start=(hh == 0),
                                    stop=(hh == HPC - 1),
                                )
                        for oh in range(2):
                            ot = pd.tile(
                                [D, QW], F32, name=f"ot{ss}_{oh}", tag="ot", bufs=4
                            )
                            nc.vector.tensor_add(
                                out=ot,
                                in0=ps_o[:, oh * QW : (oh + 1) * QW],
                                in1=bo_rep[:, oh * QW : (oh + 1) * QW],
                            )
                            nc.sync.dma_start(
                                out=out.ap()[
                                    ss * D : (ss + 1) * D, oh * QW : (oh + 1) * QW
                                ],
                                in_=ot,
                            )

    nc.compile()
    return nc


def _get_program():
    global _PROGRAM
    if _PROGRAM is None:
        _PROGRAM = _build()
    return _PROGRAM


def make_in_maps(q, k, v, Wq, bq, Wk, bk, Wv, bv, Wo, bo):
    f32 = lambda a: np.asarray(a, dtype=np.float32)
    q, k, v = f32(q), f32(k), f32(v)
    Wq, bq, Wk, bk, Wv, bv, Wo, bo = (
        f32(Wq), f32(bq), f32(Wk), f32(bk), f32(Wv), f32(bv), f32(Wo), f32(bo)
    )
    ident = np.eye(D, dtype=BF16)
    in_maps = []
    xT = {}
    for b in range(B):
        xT[b] = (
            np.ascontiguousarray(q[b].T).astype(BF16),
            np.ascontiguousarray(k[b].T).astype(BF16),
            np.ascontiguousarray(v[b].T).astype(BF16),
        )
    halves = []
    for half in range(2):
        Wo_half = Wo[half * QC : (half + 1) * QC, :]
        bv_half = bv[half * KV : (half + 1) * KV]
        bv_exp = np.concatenate(
            [bv_half[(j // 4) * D : (j // 4 + 1) * D] for j in range(HPC)]
        )
        bo_eff = (bo if half == 0 else np.zeros_like(bo)).astype(
            np.float64
        ) + bv_exp.astype(np.float64) @ Wo_half.astype(np.float64)
        halves.append(
            {
                "wq": np.ascontiguousarray(
                    Wq[:, half * QC : (half + 1) * QC]
                ).astype(BF16),
                "wk": np.ascontiguousarray(
                    Wk[:, half * KV : (half + 1) * KV]
                ).astype(BF16),
                "wv": np.ascontiguousarray(
                    Wv[:, half * KV : (half + 1) * KV]
                ).astype(BF16),
                "wo": np.ascontiguousarray(Wo_half).astype(BF16),
                "bq": np.ascontiguousarray(bq[half * QC : (half + 1) * QC]),
                "bo": bo_eff.astype(np.float32),
                "ident": ident,
            }
        )
    for c in range(N_CORES):
        b, half = c // 2, c % 2
        xqT, xkT, xvT = xT[b]
        in_maps.append({"xq": xqT, "xk": xkT, "xv": xvT, **halves[half]})
    return in_maps


def combine_results(results):
    out = np.empty((B, S, E), np.float32)
    for b in range(B):
        out[b] = np.asarray(results[2 * b]["out_p"]) + np.asarray(
            results[2 * b + 1]["out_p"]
        )
    return out


def kernel(q, k, v, Wq, bq, Wk, bk, Wv, bv, Wo, bo):
    from concourse.bass_utils import run_bass_kernel_spmd

    nc = _get_program()
    in_maps = make_in_maps(q, k, v, Wq, bq, Wk, bk, Wv, bv, Wo, bo)
    res = run_bass_kernel_spmd(nc, in_maps, core_ids=list(range(N_CORES)))
    return combine_results(res.results)
